# revision 40
# baseline (speedup 1.0000x reference)
"""Distributed multi-head attention kernel for 8 TRN2 NeuronCores.

Problem: x [4, 2048, 1024] -> qkv proj -> 16-head attention (d=64)
         -> out proj + bias -> [4, 2048, 1024].

Sharding (head-split, no collectives): core i handles batch b = i//2 and
head-half hh = i%2 (8 heads, full 2048-token sequence). Each core
computes Q/K/V projections only for its own 8 heads, attention for
those heads, and a partial output projection (+bias on hh=0 cores).
The host sums the two partial outputs per batch.

The kernel is built around the ScalarE exp() bottleneck (33.5M
exp/core; an ACTIVATE costs ~(N+312)/1.2 ns, so exp runs as 256
[128,1024] psum-span instructions at ~1.15us cadence). Everything else
is arranged to never stall ACT:

  - S^T spans are double-buffered (4 psum banks) and emitted two steps
    ahead; after each ACT the S lookahead is emitted BEFORE PV so the
    ACT-critical chain (S -> ACT) stays short.
  - U [65,1024] accumulates PV per (head, q-chunk); row 64 (ones column
    of V) gives softmax denominators free. U is drained to SBUF
    immediately to free its single psum buffer; the normalize
    (K=1 broadcast matmul of D, reciprocal_approx_fast, gpsimd
    multiply into pair-packed UN) runs lazily as filler work.
  - Projections and the output projection run as small filler closures
    (<=4 matmuls) paced by emission deadlines between attention steps.
  - Input DMAs are split across the two hardware DGE queues (SP + ACT);
    K=1 warmup matmuls keep the PE HAM clock-gate warm while they land.
  - Unit order is qc-major within a head-pair so the first token-half's
    output projection + DMA-out streams during the last units.
"""

import numpy as np
import ml_dtypes

B = 4
N = 2048
DIM = 1024
HEADS = 16
DH = 64
NCORES = 8
NH = 8       # heads per core
NPAIR = 4    # head pairs per core

_CACHE = {}


def _build_nc():
    from contextlib import ExitStack

    import concourse.bass as bass
    import concourse.mybir as mybir
    import concourse.tile as tile
    from concourse import bacc

    f32 = mybir.dt.float32
    bf16 = mybir.dt.bfloat16
    f16 = mybir.dt.float16
    EXP = mybir.ActivationFunctionType.Exp

    nc = bacc.Bacc("TRN2", target_bir_lowering=False, debug=False,
                   num_devices=NCORES)

    xt_d = nc.dram_tensor("xt", [DIM, N], bf16, kind="ExternalInput")
    wq_d = nc.dram_tensor("wq", [128, 4096], bf16, kind="ExternalInput")
    wk_d = nc.dram_tensor("wk", [128, 4096], bf16, kind="ExternalInput")
    wv_d = nc.dram_tensor("wv", [128, 4096], bf16, kind="ExternalInput")
    wo_d = nc.dram_tensor("wo", [NPAIR, 128, DIM], bf16, kind="ExternalInput")
    bias_d = nc.dram_tensor("bias", [128, DIM], bf16, kind="ExternalInput")
    out_d = nc.dram_tensor("out", [N, DIM], bf16, kind="ExternalOutput")

    with tile.TileContext(nc) as tc, ExitStack() as top:
        const_pool = top.enter_context(tc.tile_pool(name="const", bufs=1))
        s_ps = top.enter_context(tc.tile_pool(name="sps", bufs=2, space="PSUM"))
        u_ps = top.enter_context(tc.tile_pool(name="ups", bufs=1, space="PSUM"))
        mm_ps = top.enter_context(tc.tile_pool(name="mmps", bufs=2, space="PSUM"))
        es_pool = top.enter_context(tc.tile_pool(name="es", bufs=4))
        ur_pool = top.enter_context(tc.tile_pool(name="ur", bufs=4))
        d_pool = top.enter_context(tc.tile_pool(name="dsb", bufs=4))
        r_pool = top.enter_context(tc.tile_pool(name="rsb", bufs=2))
        un_pool = top.enter_context(tc.tile_pool(name="un", bufs=1))

        ones_t = const_pool.tile([1, 128], f16, tag="ones", name="ones")
        nc.gpsimd.memset(ones_t[:], 1.0)
        warm_t = const_pool.tile([1, 512], bf16, tag="warm", name="warm")
        nc.gpsimd.memset(warm_t[:], 0.0)
        bias_t = const_pool.tile([128, DIM], bf16, tag="bias", name="bias")

        # ---- static input tiles -------------------------------------
        # (xt/w innermost: released mid-kernel; pool releases are LIFO)
        qkv_pool = tc.alloc_tile_pool(name="qkv", bufs=1)
        wo_pool = tc.alloc_tile_pool(name="wo", bufs=1)
        xt_pool = tc.alloc_tile_pool(name="xt", bufs=1)
        w_pool = tc.alloc_tile_pool(name="w", bufs=1)
        xt_all = xt_pool.tile([128, 8, N], bf16, tag="xt", name="xt")
        xt = [xt_all[:, i, :] for i in range(8)]
        wq_t = w_pool.tile([128, 4096], bf16, tag="wq", name="wq")
        wk_t = w_pool.tile([128, 4096], bf16, tag="wk", name="wk")
        wv_t = w_pool.tile([128, 4096], bf16, tag="wv", name="wv")
        wq = [wq_t[:, i * 512:(i + 1) * 512] for i in range(8)]
        wk = [wk_t[:, i * 512:(i + 1) * 512] for i in range(8)]
        wv = [wv_t[:, i * 512:(i + 1) * 512] for i in range(8)]
        WO = [wo_pool.tile([128, DIM], bf16, tag=f"wo{p}", name=f"wo{p}")
              for p in range(NPAIR)]

        # DMA split across the two HWDGE queues: SP gets what the first
        # projections need (wk, xt first halves); ACT gets the rest.
        # priority order: the preamble needs wk+xtA (sync) and wv+wq
        # (scalar) only; xtB next; wo/bias are needed ~2/3 in.
        nc.sync.dma_start(wk_t[:], wk_d.ap()[:])
        xt_src = xt_d.ap().rearrange("(i p) n -> p i n", p=128)
        nc.sync.dma_start(xt_all[:, :, 0:1024], xt_src[:, :, 0:1024])
        nc.sync.dma_start(xt_all[:, :, 1024:2048], xt_src[:, :, 1024:2048])
        for p in range(NPAIR):
            nc.sync.dma_start(WO[p][:], wo_d.ap()[p])
        nc.sync.dma_start(bias_t[:], bias_d.ap()[:])
        nc.scalar.dma_start(wv_t[:], wv_d.ap()[:])
        nc.scalar.dma_start(wq_t[:], wq_d.ap()[:])

        QT = [qkv_pool.tile([128, N], bf16, tag=f"q{p}", name=f"q{p}")
              for p in range(NPAIR)]
        KT = [qkv_pool.tile([128, N], bf16, tag=f"k{p}", name=f"k{p}")
              for p in range(NPAIR)]
        VT = [qkv_pool.tile([128, NH, 65], bf16, tag=f"v{tb}", name=f"v{tb}")
              for tb in range(16)]
        UN = [un_pool.tile([128, N], bf16, tag=f"un{p}", name=f"un{p}")
              for p in range(NPAIR)]

        # ---- PE warmup: keep HAM at K=8/8 while input DMAs land -----
        for i in range(28):
            ps = mm_ps.tile([1, 512], f32, tag="mm", name="wu")
            nc.tensor.matmul(ps[:], ones_t[:, 0:1], warm_t[:],
                             start=True, stop=True)

        # ---- projection unit closures (split into <=4-MM halves) ----
        pending_kq = {p: 0 for p in range(NPAIR)}
        pending_v = {tb: 0 for tb in range(16)}

        def kq_first(box, w, p, t):
            ps = mm_ps.tile([128, 512], f32, tag="mm", name="mm")
            box[0] = ps
            for fc in range(4):
                nc.tensor.matmul(
                    ps[:], w[fc][:, p * 128:(p + 1) * 128],
                    xt[fc][:, t * 512:(t + 1) * 512],
                    start=(fc == 0), stop=False)

        def kq_second(box, dest, w, p, t):
            ps = box[0]
            for fc in range(4, 8):
                nc.tensor.matmul(
                    ps[:], w[fc][:, p * 128:(p + 1) * 128],
                    xt[fc][:, t * 512:(t + 1) * 512],
                    start=False, stop=(fc == 7))
            nc.vector.tensor_copy(dest[p][:, t * 512:(t + 1) * 512], ps[:])

        def kq_unit(dest, w, p, t):
            box = [None]
            kq_first(box, w, p, t)
            kq_second(box, dest, w, p, t)

        def v_first(box, tb):
            ps = mm_ps.tile([128, 512], f32, tag="mm", name="mm")
            box[0] = ps
            for fc in range(4):
                nc.tensor.matmul(
                    ps[:], xt[fc][:, tb * 128:(tb + 1) * 128], wv[fc][:],
                    start=(fc == 0), stop=False)

        def v_second(box, tb):
            ps = box[0]
            for fc in range(4, 8):
                nc.tensor.matmul(
                    ps[:], xt[fc][:, tb * 128:(tb + 1) * 128], wv[fc][:],
                    start=False, stop=(fc == 7))
            nc.vector.tensor_copy(
                VT[tb][:, :, 0:64],
                ps[:].rearrange("p (h d) -> p h d", d=64))
            nc.gpsimd.memset(VT[tb][:, :, 64:65], 1.0)

        def v_unit(tb):
            box = [None]
            v_first(box, tb)
            v_second(box, tb)

        # ---- fillers with emission deadlines ------------------------
        fillers = []
        state = {"emitted": 0, "total": 0}

        def add_filler(latest, fn):
            fillers.append((latest, fn))

        def pop_filler():
            _, fn = fillers.pop(0)
            fn()
            state["emitted"] += 1

        def add_kq_filler(latest, dest, w, p, t):
            box = [None]
            pending_kq[p] += 2

            def first():
                kq_first(box, w, p, t)
                pending_kq[p] -= 1

            def second():
                kq_second(box, dest, w, p, t)
                pending_kq[p] -= 1

            add_filler(latest, first)
            add_filler(latest, second)

        def add_v_filler(latest, tb):
            box = [None]
            pending_v[tb] += 2

            def first():
                v_first(box, tb)
                pending_v[tb] -= 1

            def second():
                v_second(box, tb)
                pending_v[tb] -= 1

            add_filler(latest, first)
            add_filler(latest, second)

        def maybe_fill(done, steps):
            # deadlines are correctness-critical (a write filler emitted
            # after its reader leaves the reader on stale data): pop
            # through the last due entry; then pace (max 3 per call).
            # deadline-driven only: the spread deadlines already encode a
            # near-uniform ~1 closure / 3 steps feed rate, which keeps
            # the PE ~95% busy (warm HAM clock) without starving ACT.
            due_idx = -1
            for i, (latest, _) in enumerate(fillers):
                if latest is not None and done >= latest:
                    due_idx = i
            while due_idx >= 0:
                pop_filler()
                due_idx -= 1

        def force_kq(p):
            while pending_kq[p] > 0:
                pop_filler()

        def force_v(tb):
            while pending_v[tb] > 0:
                pop_filler()

        # preamble: what (pair 0, qc 0) needs before its first steps
        kq_unit(KT, wk, 0, 0)
        kq_unit(QT, wq, 0, 0)
        kq_unit(QT, wq, 0, 1)
        for tb in range(2):
            v_unit(tb)

        # remaining proj as deadline fillers
        add_kq_filler(0, KT, wk, 0, 1)
        for tb in range(2, 16):
            add_v_filler(max(0, tb - 4), tb)
        add_kq_filler(5, KT, wk, 0, 2)
        add_kq_filler(9, KT, wk, 0, 3)
        for t in range(2, 4):
            add_kq_filler(16, QT, wq, 0, t)
        for p in range(1, NPAIR):
            base = 64 * (p - 1) + 16
            for i, (dest, w, t) in enumerate(
                    [(KT, wk, t) for t in range(4)] +
                    [(QT, wq, t) for t in range(4)]):
                add_kq_filler(base + 5 * i, dest, w, p, t)

        # ---- attention, software-pipelined across all 16 units ------
        # qc-major within a pair: both heads' qc=0 first, so the first
        # token-half's output can stream during the last units.
        units = [(p, hh, qc) for p in range(NPAIR) for qc in range(2)
                 for hh in range(2)]
        NU = len(units)
        GTOT = NU * 16

        def emit_S(gidx):
            u, kc = divmod(gidx, 16)
            p, hh, qc = units[u]
            if kc == 0:
                force_kq(p)
            hb = hh * 64
            st = s_ps.tile([128, 1024], f32, tag="s", name="s")
            for j in range(2):
                nc.tensor.matmul(
                    st[:, j * 512:(j + 1) * 512],
                    KT[p][hb:hb + 64, kc * 128:(kc + 1) * 128],
                    QT[p][hb:hb + 64,
                          qc * 1024 + j * 512:qc * 1024 + j * 512 + 512],
                    start=True, stop=True)
            return st

        def norm_rest(p, hh, qc, ur, dsb):
            """Lazy normalize: broadcast 1/D and multiply into UN."""
            hb = hh * 64
            for j in range(2):
                bc = mm_ps.tile([128, 512], f32, tag="mm", name="bc")
                nc.tensor.matmul(bc[:], ones_t[:],
                                 dsb[:, j * 512:(j + 1) * 512],
                                 start=True, stop=True)
                rs = r_pool.tile([64, 512], f32, tag="rs", name="rs")
                nc.vector.reciprocal_approx_fast(rs[:], bc[0:64, :])
                nc.gpsimd.tensor_mul(
                    UN[p][hb:hb + 64,
                          qc * 1024 + j * 512:qc * 1024 + j * 512 + 512],
                    ur[:, j * 512:(j + 1) * 512], rs[:])

        S_tiles = {0: emit_S(0), 1: emit_S(1)}
        U_box = [None]

        passA_added = [False, False]
        passB_added = [False]
        fin_state = {}

        def setup_fin():
            w_pool.release()
            xt_pool.release()
            fin_state["pool"] = tc.alloc_tile_pool(name="fin", bufs=1)
            fin_state["FIN"] = [
                fin_state["pool"].tile([128, DIM], bf16, tag=f"fin{qf}",
                                       name=f"fin{qf}")
                for qf in range(16)]

        def passA(qf, of):
            FIN = fin_state["FIN"]
            ps = mm_ps.tile([128, 512], f32, tag="mm", name="pa")
            for p in range(3):
                nc.tensor.matmul(
                    ps[:], UN[p][:, qf * 128:(qf + 1) * 128],
                    WO[p][:, of * 512:(of + 1) * 512],
                    start=(p == 0), stop=(p == 2))
            nc.vector.tensor_add(
                FIN[qf][:, of * 512:(of + 1) * 512], ps[:],
                bias_t[:, of * 512:(of + 1) * 512])

        def passB_of(qf, of):
            FIN = fin_state["FIN"]
            ps = mm_ps.tile([128, 512], f32, tag="mm", name="pb")
            nc.tensor.matmul(
                ps[:], UN[3][:, qf * 128:(qf + 1) * 128],
                WO[3][:, of * 512:(of + 1) * 512],
                start=True, stop=True)
            nc.vector.tensor_add(
                FIN[qf][:, of * 512:(of + 1) * 512],
                FIN[qf][:, of * 512:(of + 1) * 512], ps[:])

        def passB_dma(qf):
            nc.sync.dma_start(out_d.ap()[qf * 128:(qf + 1) * 128, :],
                              fin_state["FIN"][qf][:])

        def passB(qf):
            passB_of(qf, 0)
            passB_of(qf, 1)
            passB_dma(qf)

        es_tiles = {}

        def pv_job(gp, gnow):
            """PV for step gp (runs 2 steps late so a late V tile never
            blocks the ACT-critical S chain in the in-order PE queue)."""
            u, kc = divmod(gp, 16)
            p, hh, qc = units[u]
            hloc = 2 * p + hh
            es = es_tiles.pop(gp)
            if kc == 0:
                U_box[0] = u_ps.tile([65, 1024], f32, tag="u", name="u")
            U = U_box[0]
            force_v(kc)
            for j in range(2):
                nc.tensor.matmul(
                    U[:, j * 512:(j + 1) * 512],
                    VT[kc][:, hloc, 0:65],
                    es[:, j * 512:(j + 1) * 512],
                    start=(kc == 0), stop=(kc == 15))
            if kc == 15:
                # fast U drain: free the single U psum buffer ASAP
                ur = ur_pool.tile([64, 1024], bf16, tag="ur", name="ur")
                nc.vector.tensor_copy(ur[:], U[0:64, :])
                dsb = d_pool.tile([1, 1024], f16, tag="d", name="d")
                with nc.allow_low_precision(reason="softmax denom f16"):
                    nc.vector.tensor_copy(dsb[:], U[64:65, :])
                add_filler(min(gnow + 14, 250),
                           lambda p=p, hh=hh, qc=qc, ur=ur, dsb=dsb:
                           norm_rest(p, hh, qc, ur, dsb))
                if u == 9 and not passA_added[0]:
                    # pairs 0-2 qc0 done: out-proj for tokens 0-1023
                    passA_added[0] = True
                    setup_fin()
                    for i, (qf, of) in enumerate(
                            (qf, of) for qf in range(8) for of in range(2)):
                        add_filler(gnow + 16 + i * 2,
                                   lambda qf=qf, of=of: passA(qf, of))
                if u == 11 and not passA_added[1]:
                    passA_added[1] = True
                    for i, (qf, of) in enumerate(
                            (qf, of) for qf in range(8, 16) for of in range(2)):
                        add_filler(gnow + 16 + i * 2,
                                   lambda qf=qf, of=of: passA(qf, of))
                if u == 13 and not passB_added[0]:
                    passB_added[0] = True
                    i = 0
                    for qf in range(8):
                        for of in range(2):
                            add_filler(gnow + 15 + i,
                                       lambda qf=qf, of=of: passB_of(qf, of))
                            i += 1
                        add_filler(gnow + 15 + i,
                                   lambda qf=qf: passB_dma(qf))

        for gidx in range(GTOT):
            st = S_tiles.pop(gidx)
            es = es_pool.tile([128, 1024], bf16, tag="es", name="es")
            nc.scalar.activation(es[:], st[:], EXP, scale=0.125)
            es_tiles[gidx] = es
            # S lookahead FIRST: it feeds ACT(g+2), the critical chain.
            if gidx + 2 < GTOT:
                S_tiles[gidx + 2] = emit_S(gidx + 2)
            if gidx - 2 >= 0:
                pv_job(gidx - 2, gidx)
            maybe_fill(gidx, GTOT)

        pv_job(GTOT - 2, GTOT - 1)
        pv_job(GTOT - 1, GTOT - 1)

        # flush remaining fillers (incl. last norms and any passA/B)
        while fillers:
            pop_filler()

        # tail: second token-half out-proj + DMA
        for qf in range(8, 16):
            passB(qf)

        fin_state["pool"].release()
        wo_pool.release()
        qkv_pool.release()

    nc.compile()
    return nc


def _get_nc():
    if "nc" not in _CACHE:
        _CACHE["nc"] = _build_nc()
    return _CACHE["nc"]


def _make_in_maps(x, w_qkv, w_out, b_out):
    bf = ml_dtypes.bfloat16

    def wslice(w, hh):
        # [1024, 512] -> [128, 8, 512] (partition p holds w[fc*128+p, :]
        # at slot fc) -> [128, 4096]
        s = np.asarray(w[:, hh * 512:(hh + 1) * 512], np.float32)
        return np.ascontiguousarray(
            s.reshape(8, 128, 512).transpose(1, 0, 2).reshape(128, 4096)
        ).astype(bf)

    xts = [np.ascontiguousarray(np.asarray(x[b], np.float32).T).astype(bf)
           for b in range(B)]
    wq_f = w_qkv[:, 0:1024]
    wk_f = w_qkv[:, 1024:2048]
    wv_f = w_qkv[:, 2048:3072]
    wo_f = np.asarray(w_out, np.float32)  # [1024 inner, 1024 out]
    bias_rep = np.broadcast_to(
        np.asarray(b_out, np.float32).reshape(1, DIM), (128, DIM))
    zeros = np.zeros((128, DIM), np.float32)
    in_maps = []
    for i in range(NCORES):
        b, hh = i // 2, i % 2
        wo_core = np.ascontiguousarray(
            wo_f[hh * 512:(hh + 1) * 512, :]).reshape(NPAIR, 128, DIM)
        in_maps.append({
            "xt": xts[b],
            "wq": wslice(wq_f, hh),
            "wk": wslice(wk_f, hh),
            "wv": wslice(wv_f, hh),
            "wo": wo_core.astype(bf),
            "bias": np.ascontiguousarray(
                (bias_rep if hh == 0 else zeros)).astype(bf),
        })
    return in_maps


def _assemble(results):
    out = np.empty((B, N, DIM), np.float32)
    for b in range(B):
        out[b] = (results[2 * b]["out"].astype(np.float32) +
                  results[2 * b + 1]["out"].astype(np.float32))
    return out


def run(x, w_qkv, w_out, b_out, trace=False):
    """Run the kernel; returns (output, BassKernelResults)."""
    from concourse.bass_utils import run_bass_kernel_spmd
    nc = _get_nc()
    in_maps = _make_in_maps(x, w_qkv, w_out, b_out)
    res = run_bass_kernel_spmd(nc, in_maps, core_ids=list(range(NCORES)),
                               trace=trace)
    return _assemble(res.results), res


def kernel(x, w_qkv, w_out, b_out):
    out, _ = run(x, w_qkv, w_out, b_out, trace=False)
    return out


# revision 52
# speedup vs baseline: 1.2605x; 1.2605x over previous
"""Distributed multi-head attention kernel for 8 TRN2 NeuronCores.

Problem: x [4, 2048, 1024] -> qkv proj -> 16-head attention (d=64)
         -> out proj + bias -> [4, 2048, 1024].

Sharding (head-split, no collectives): core i handles batch b = i//2 and
head-half hh = i%2 (8 heads, full 2048-token sequence). Each core
computes Q/K/V projections only for its own 8 heads, attention for
those heads, and a partial output projection (+bias on hh=0 cores).
The host sums the two partial outputs per batch.

The kernel is built around the ScalarE exp() bottleneck (33.5M
exp/core; an ACTIVATE costs ~(N+312)/1.2 ns, so exp runs as 256
[128,1024] psum-span instructions at ~1.15us cadence). Everything else
is arranged to never stall ACT:

  - S^T spans are double-buffered (4 psum banks) and emitted two steps
    ahead; after each ACT the S lookahead is emitted BEFORE PV so the
    ACT-critical chain (S -> ACT) stays short.
  - U [65,1024] accumulates PV per (head, q-chunk); row 64 (ones column
    of V) gives softmax denominators free. U is drained to SBUF
    immediately to free its single psum buffer; the normalize
    (K=1 broadcast matmul of D, reciprocal_approx_fast, gpsimd
    multiply into pair-packed UN) runs lazily as filler work.
  - Projections and the output projection run as small filler closures
    (<=4 matmuls) paced by emission deadlines between attention steps.
  - Input DMAs are split across the two hardware DGE queues (SP + ACT);
    K=1 warmup matmuls keep the PE HAM clock-gate warm while they land.
  - Unit order is qc-major within a head-pair so the first token-half's
    output projection + DMA-out streams during the last units.
"""

import numpy as np
import ml_dtypes

B = 4
N = 2048
DIM = 1024
HEADS = 16
DH = 64
NCORES = 8
NH = 8       # heads per core
NPAIR = 4    # head pairs per core

_CACHE = {}


def _build_nc():
    from contextlib import ExitStack

    import concourse.bass as bass
    import concourse.mybir as mybir
    import concourse.tile as tile
    from concourse import bacc

    f32 = mybir.dt.float32
    bf16 = mybir.dt.bfloat16
    f16 = mybir.dt.float16
    EXP = mybir.ActivationFunctionType.Exp

    nc = bacc.Bacc("TRN2", target_bir_lowering=False, debug=False,
                   num_devices=NCORES)

    xt_d = nc.dram_tensor("xt", [DIM, N], bf16, kind="ExternalInput")
    wq_d = nc.dram_tensor("wq", [128, 4096], bf16, kind="ExternalInput")
    wk_d = nc.dram_tensor("wk", [128, 4096], bf16, kind="ExternalInput")
    wv_d = nc.dram_tensor("wv", [128, 4096], bf16, kind="ExternalInput")
    wo_d = nc.dram_tensor("wo", [NPAIR, 128, DIM], bf16, kind="ExternalInput")
    bias_d = nc.dram_tensor("bias", [128, DIM], bf16, kind="ExternalInput")
    out_d = nc.dram_tensor("out", [N, DIM], bf16, kind="ExternalOutput")

    with tile.TileContext(nc) as tc, ExitStack() as top:
        const_pool = top.enter_context(tc.tile_pool(name="const", bufs=1))
        s_ps = top.enter_context(tc.tile_pool(name="sps", bufs=2, space="PSUM"))
        u_ps = top.enter_context(tc.tile_pool(name="ups", bufs=1, space="PSUM"))
        mm_ps = top.enter_context(tc.tile_pool(name="mmps", bufs=2, space="PSUM"))
        es_pool = top.enter_context(tc.tile_pool(name="es", bufs=6))
        ur_pool = top.enter_context(tc.tile_pool(name="ur", bufs=4))
        d_pool = top.enter_context(tc.tile_pool(name="dsb", bufs=4))
        r_pool = top.enter_context(tc.tile_pool(name="rsb", bufs=2))
        un_pool = top.enter_context(tc.tile_pool(name="un", bufs=1))

        ones_t = const_pool.tile([1, 128], f16, tag="ones", name="ones")
        nc.gpsimd.memset(ones_t[:], 1.0)
        warm_t = const_pool.tile([128, 512], bf16, tag="warm", name="warm")
        nc.gpsimd.memset(warm_t[:], 0.0)
        bias_t = const_pool.tile([128, DIM], bf16, tag="bias", name="bias")

        # ---- static input tiles -------------------------------------
        # (xt/w innermost: released mid-kernel; pool releases are LIFO)
        qkv_pool = tc.alloc_tile_pool(name="qkv", bufs=1)
        wo_pool = tc.alloc_tile_pool(name="wo", bufs=1)
        xt_pool = tc.alloc_tile_pool(name="xt", bufs=1)
        w_pool = tc.alloc_tile_pool(name="w", bufs=1)
        xt_all = xt_pool.tile([128, 8, N], bf16, tag="xt", name="xt")
        xt = [xt_all[:, i, :] for i in range(8)]
        wq_t = w_pool.tile([128, 4096], bf16, tag="wq", name="wq")
        wk_t = w_pool.tile([128, 4096], bf16, tag="wk", name="wk")
        wv_t = w_pool.tile([128, 4096], bf16, tag="wv", name="wv")
        wq = [wq_t[:, i * 512:(i + 1) * 512] for i in range(8)]
        wk = [wk_t[:, i * 512:(i + 1) * 512] for i in range(8)]
        wv = [wv_t[:, i * 512:(i + 1) * 512] for i in range(8)]
        WO = [wo_pool.tile([128, DIM], bf16, tag=f"wo{p}", name=f"wo{p}")
              for p in range(NPAIR)]

        # DMA split across the two HWDGE queues: SP gets what the first
        # projections need (wk, xt first halves); ACT gets the rest.
        # priority order: the preamble needs wk+xtA (sync) and wv+wq
        # (scalar) only; xtB next; wo/bias are needed ~2/3 in.
        nc.sync.dma_start(wk_t[:], wk_d.ap()[:])
        xt_src = xt_d.ap().rearrange("(i p) n -> p i n", p=128)
        for qtr in range(4):
            nc.sync.dma_start(xt_all[:, :, qtr * 512:(qtr + 1) * 512],
                              xt_src[:, :, qtr * 512:(qtr + 1) * 512])
        for p in range(NPAIR):
            nc.sync.dma_start(WO[p][:], wo_d.ap()[p])
        nc.sync.dma_start(bias_t[:], bias_d.ap()[:])
        nc.scalar.dma_start(wq_t[:], wq_d.ap()[:])
        nc.scalar.dma_start(wv_t[:], wv_d.ap()[:])

        QT = [qkv_pool.tile([128, N], bf16, tag=f"q{p}", name=f"q{p}")
              for p in range(NPAIR)]
        KT = [qkv_pool.tile([128, N], bf16, tag=f"k{p}", name=f"k{p}")
              for p in range(NPAIR)]
        VT = [qkv_pool.tile([128, NH, 65], bf16, tag=f"v{tb}", name=f"v{tb}")
              for tb in range(16)]
        UN = [un_pool.tile([128, N], bf16, tag=f"un{p}", name=f"un{p}")
              for p in range(NPAIR)]

        # ---- PE warmup: keep HAM at K=8/8 while input DMAs land -----
        # dense K=128 matmuls: K=1 streams don't register enough activity
        # to lift the HAM clock gate.
        for i in range(40):
            ps = mm_ps.tile([128, 512], f32, tag="mm", name="wu")
            nc.tensor.matmul(ps[:], warm_t[:, 0:128], warm_t[:],
                             start=True, stop=True)

        # ---- projection unit closures (split into <=4-MM halves) ----
        # pending counters are per (tensor, pair, tchunk) so a force only
        # pulls exactly what an S step needs, not the whole pair.
        pending_kq = {}
        pending_v = {tb: 0 for tb in range(16)}

        def kq_first(box, w, p, t):
            ps = mm_ps.tile([128, 512], f32, tag="mm", name="mm")
            box[0] = ps
            for fc in range(4):
                nc.tensor.matmul(
                    ps[:], w[fc][:, p * 128:(p + 1) * 128],
                    xt[fc][:, t * 512:(t + 1) * 512],
                    start=(fc == 0), stop=False)

        def kq_second(box, dest, w, p, t):
            ps = box[0]
            for fc in range(4, 8):
                nc.tensor.matmul(
                    ps[:], w[fc][:, p * 128:(p + 1) * 128],
                    xt[fc][:, t * 512:(t + 1) * 512],
                    start=False, stop=(fc == 7))
            nc.vector.tensor_copy(dest[p][:, t * 512:(t + 1) * 512], ps[:])

        def kq_unit(dest, w, p, t):
            box = [None]
            kq_first(box, w, p, t)
            kq_second(box, dest, w, p, t)

        def v_first(box, tb):
            ps = mm_ps.tile([128, 512], f32, tag="mm", name="mm")
            box[0] = ps
            for fc in range(4):
                nc.tensor.matmul(
                    ps[:], xt[fc][:, tb * 128:(tb + 1) * 128], wv[fc][:],
                    start=(fc == 0), stop=False)

        def v_second(box, tb):
            ps = box[0]
            for fc in range(4, 8):
                nc.tensor.matmul(
                    ps[:], xt[fc][:, tb * 128:(tb + 1) * 128], wv[fc][:],
                    start=False, stop=(fc == 7))
            nc.vector.tensor_copy(
                VT[tb][:, :, 0:64],
                ps[:].rearrange("p (h d) -> p h d", d=64))
            nc.gpsimd.memset(VT[tb][:, :, 64:65], 1.0)

        def v_unit(tb):
            box = [None]
            v_first(box, tb)
            v_second(box, tb)

        # ---- fillers with emission deadlines ------------------------
        fillers = []
        state = {"emitted": 0, "total": 0}

        def add_filler(latest, fn):
            fillers.append((latest, fn))

        def pop_filler():
            _, fn = fillers.pop(0)
            fn()
            state["emitted"] += 1

        def add_kq_filler(latest, dest, w, p, t):
            box = [None]
            key = (id(dest), p, t)
            pending_kq[key] = pending_kq.get(key, 0) + 2

            def first():
                kq_first(box, w, p, t)
                pending_kq[key] -= 1

            def second():
                kq_second(box, dest, w, p, t)
                pending_kq[key] -= 1

            add_filler(latest, first)
            add_filler(latest, second)

        def add_v_filler(latest, tb):
            box = [None]
            pending_v[tb] += 2

            def first():
                v_first(box, tb)
                pending_v[tb] -= 1

            def second():
                v_second(box, tb)
                pending_v[tb] -= 1

            add_filler(latest, first)
            add_filler(latest, second)

        def maybe_fill(done, steps):
            # deadlines are correctness-critical (a write filler emitted
            # after its reader leaves the reader on stale data): pop
            # through the last due entry; then pace (max 3 per call).
            # deadline-driven only: the spread deadlines already encode a
            # near-uniform ~1 closure / 3 steps feed rate, which keeps
            # the PE ~95% busy (warm HAM clock) without starving ACT.
            due_idx = -1
            for i, (latest, _) in enumerate(fillers):
                if latest is not None and done >= latest:
                    due_idx = i
            while due_idx >= 0:
                pop_filler()
                due_idx -= 1

        def force_keys(keys):
            while any(pending_kq.get(k, 0) > 0 for k in keys):
                pop_filler()

        def force_v(tb):
            while pending_v[tb] > 0:
                pop_filler()

        # preamble: what (pair 0, qc 0) needs before its first steps
        kq_unit(KT, wk, 0, 0)
        kq_unit(QT, wq, 0, 0)
        kq_unit(QT, wq, 0, 1)
        for tb in range(2):
            v_unit(tb)

        # remaining proj as deadline fillers
        add_kq_filler(0, KT, wk, 0, 1)
        for tb in range(2, 16):
            add_v_filler(max(0, tb - 4), tb)
        add_kq_filler(5, KT, wk, 0, 2)
        add_kq_filler(9, KT, wk, 0, 3)
        for t in range(2, 4):
            add_kq_filler(16, QT, wq, 0, t)
        for p in range(1, NPAIR):
            base = 64 * (p - 1) + 16
            for i, (dest, w, t) in enumerate(
                    [(KT, wk, t) for t in range(4)] +
                    [(QT, wq, t) for t in range(4)]):
                add_kq_filler(base + 5 * i, dest, w, p, t)

        # ---- attention, software-pipelined across all 16 units ------
        # qc-major within a pair: both heads' qc=0 first, so the first
        # token-half's output can stream during the last units.
        units = [(p, hh, qc) for p in range(NPAIR) for qc in range(2)
                 for hh in range(2)]
        NU = len(units)
        GTOT = NU * 16

        def emit_S(gidx):
            u, kc = divmod(gidx, 16)
            p, hh, qc = units[u]
            force_keys([(id(KT), p, kc // 4),
                        (id(QT), p, 2 * qc), (id(QT), p, 2 * qc + 1)])
            hb = hh * 64
            st = s_ps.tile([128, 1024], f32, tag="s", name="s")
            for j in range(2):
                nc.tensor.matmul(
                    st[:, j * 512:(j + 1) * 512],
                    KT[p][hb:hb + 64, kc * 128:(kc + 1) * 128],
                    QT[p][hb:hb + 64,
                          qc * 1024 + j * 512:qc * 1024 + j * 512 + 512],
                    start=True, stop=True)
            return st

        def norm_rest(p, hh, qc, ur, dsb):
            """Lazy normalize: broadcast 1/D and multiply into UN."""
            hb = hh * 64
            for j in range(2):
                bc = mm_ps.tile([128, 512], f32, tag="mm", name="bc")
                nc.tensor.matmul(bc[:], ones_t[:],
                                 dsb[:, j * 512:(j + 1) * 512],
                                 start=True, stop=True)
                rs = r_pool.tile([64, 512], f32, tag="rs", name="rs")
                nc.vector.reciprocal_approx_fast(rs[:], bc[0:64, :])
                nc.gpsimd.tensor_mul(
                    UN[p][hb:hb + 64,
                          qc * 1024 + j * 512:qc * 1024 + j * 512 + 512],
                    ur[:, j * 512:(j + 1) * 512], rs[:])

        S_tiles = {0: emit_S(0), 1: emit_S(1)}
        U_box = [None]

        passA_added = [False, False]
        passB_added = [False]
        fin_state = {}

        def setup_fin():
            w_pool.release()
            xt_pool.release()
            fin_state["pool"] = tc.alloc_tile_pool(name="fin", bufs=1)
            fin_state["FIN"] = [
                fin_state["pool"].tile([128, DIM], bf16, tag=f"fin{qf}",
                                       name=f"fin{qf}")
                for qf in range(16)]

        def passA(qf, of):
            FIN = fin_state["FIN"]
            ps = mm_ps.tile([128, 512], f32, tag="mm", name="pa")
            for p in range(3):
                nc.tensor.matmul(
                    ps[:], UN[p][:, qf * 128:(qf + 1) * 128],
                    WO[p][:, of * 512:(of + 1) * 512],
                    start=(p == 0), stop=(p == 2))
            nc.vector.tensor_add(
                FIN[qf][:, of * 512:(of + 1) * 512], ps[:],
                bias_t[:, of * 512:(of + 1) * 512])

        def passB_of(qf, of):
            FIN = fin_state["FIN"]
            ps = mm_ps.tile([128, 512], f32, tag="mm", name="pb")
            nc.tensor.matmul(
                ps[:], UN[3][:, qf * 128:(qf + 1) * 128],
                WO[3][:, of * 512:(of + 1) * 512],
                start=True, stop=True)
            nc.vector.tensor_add(
                FIN[qf][:, of * 512:(of + 1) * 512],
                FIN[qf][:, of * 512:(of + 1) * 512], ps[:])

        def passB_dma(qf):
            nc.sync.dma_start(out_d.ap()[qf * 128:(qf + 1) * 128, :],
                              fin_state["FIN"][qf][:])

        def passB(qf):
            passB_of(qf, 0)
            passB_of(qf, 1)
            passB_dma(qf)

        es_tiles = {}

        def pv_job(gp, gnow):
            """PV for step gp (runs 2 steps late so a late V tile never
            blocks the ACT-critical S chain in the in-order PE queue)."""
            u, kc = divmod(gp, 16)
            p, hh, qc = units[u]
            hloc = 2 * p + hh
            es = es_tiles.pop(gp)
            if kc == 0:
                U_box[0] = u_ps.tile([65, 1024], f32, tag="u", name="u")
            U = U_box[0]
            force_v(kc)
            for j in range(2):
                nc.tensor.matmul(
                    U[:, j * 512:(j + 1) * 512],
                    VT[kc][:, hloc, 0:65],
                    es[:, j * 512:(j + 1) * 512],
                    start=(kc == 0), stop=(kc == 15))
            if kc == 15:
                # fast U drain: D row first (it gates the bc->recip->mul
                # chain), then the U rows; frees the U psum buffer ASAP.
                dsb = d_pool.tile([1, 1024], f16, tag="d", name="d")
                with nc.allow_low_precision(reason="softmax denom f16"):
                    nc.vector.tensor_copy(dsb[:], U[64:65, :])
                ur = ur_pool.tile([64, 1024], bf16, tag="ur", name="ur")
                nc.vector.tensor_copy(ur[:], U[0:64, :])
                add_filler(min(gnow + 10, 250),
                           lambda p=p, hh=hh, qc=qc, ur=ur, dsb=dsb:
                           norm_rest(p, hh, qc, ur, dsb))
                if u == 9 and not passA_added[0]:
                    # pairs 0-2 qc0 done: out-proj for tokens 0-1023
                    passA_added[0] = True
                    setup_fin()
                    for i, (qf, of) in enumerate(
                            (qf, of) for qf in range(8) for of in range(2)):
                        add_filler(gnow + 12 + i * 3,
                                   lambda qf=qf, of=of: passA(qf, of))
                if u == 11 and not passA_added[1]:
                    passA_added[1] = True
                    for i, (qf, of) in enumerate(
                            (qf, of) for qf in range(8, 16) for of in range(2)):
                        add_filler(gnow + 12 + i * 2,
                                   lambda qf=qf, of=of: passA(qf, of))
                if u == 13 and not passB_added[0]:
                    passB_added[0] = True
                    i = 0
                    for qf in range(8):
                        for of in range(2):
                            add_filler(gnow + 12 + i,
                                       lambda qf=qf, of=of: passB_of(qf, of))
                            i += 1
                        add_filler(gnow + 12 + i,
                                   lambda qf=qf: passB_dma(qf))

        for gidx in range(GTOT):
            st = S_tiles.pop(gidx)
            es = es_pool.tile([128, 1024], bf16, tag="es", name="es")
            nc.scalar.activation(es[:], st[:], EXP, scale=0.125)
            es_tiles[gidx] = es
            # S lookahead FIRST: it feeds ACT(g+2), the critical chain.
            if gidx + 2 < GTOT:
                S_tiles[gidx + 2] = emit_S(gidx + 2)
            if gidx - 2 >= 0:
                pv_job(gidx - 2, gidx)
            maybe_fill(gidx, GTOT)

        pv_job(GTOT - 2, GTOT - 1)
        pv_job(GTOT - 1, GTOT - 1)

        # flush remaining fillers (incl. last norms and any passA/B)
        while fillers:
            pop_filler()

        # tail: second token-half out-proj + DMA
        for qf in range(8, 16):
            passB(qf)

        fin_state["pool"].release()
        wo_pool.release()
        qkv_pool.release()

    nc.compile()
    return nc


def _get_nc():
    if "nc" not in _CACHE:
        _CACHE["nc"] = _build_nc()
    return _CACHE["nc"]


def _make_in_maps(x, w_qkv, w_out, b_out):
    bf = ml_dtypes.bfloat16

    def wslice(w, hh):
        # [1024, 512] -> [128, 8, 512] (partition p holds w[fc*128+p, :]
        # at slot fc) -> [128, 4096]
        s = np.asarray(w[:, hh * 512:(hh + 1) * 512], np.float32)
        return np.ascontiguousarray(
            s.reshape(8, 128, 512).transpose(1, 0, 2).reshape(128, 4096)
        ).astype(bf)

    xts = [np.ascontiguousarray(np.asarray(x[b], np.float32).T).astype(bf)
           for b in range(B)]
    wq_f = w_qkv[:, 0:1024]
    wk_f = w_qkv[:, 1024:2048]
    wv_f = w_qkv[:, 2048:3072]
    wo_f = np.asarray(w_out, np.float32)  # [1024 inner, 1024 out]
    bias_rep = np.broadcast_to(
        np.asarray(b_out, np.float32).reshape(1, DIM), (128, DIM))
    zeros = np.zeros((128, DIM), np.float32)
    in_maps = []
    for i in range(NCORES):
        b, hh = i // 2, i % 2
        wo_core = np.ascontiguousarray(
            wo_f[hh * 512:(hh + 1) * 512, :]).reshape(NPAIR, 128, DIM)
        in_maps.append({
            "xt": xts[b],
            "wq": wslice(wq_f, hh),
            "wk": wslice(wk_f, hh),
            "wv": wslice(wv_f, hh),
            "wo": wo_core.astype(bf),
            "bias": np.ascontiguousarray(
                (bias_rep if hh == 0 else zeros)).astype(bf),
        })
    return in_maps


def _assemble(results):
    out = np.empty((B, N, DIM), np.float32)
    for b in range(B):
        out[b] = (results[2 * b]["out"].astype(np.float32) +
                  results[2 * b + 1]["out"].astype(np.float32))
    return out


def run(x, w_qkv, w_out, b_out, trace=False):
    """Run the kernel; returns (output, BassKernelResults)."""
    from concourse.bass_utils import run_bass_kernel_spmd
    nc = _get_nc()
    in_maps = _make_in_maps(x, w_qkv, w_out, b_out)
    res = run_bass_kernel_spmd(nc, in_maps, core_ids=list(range(NCORES)),
                               trace=trace)
    return _assemble(res.results), res


def kernel(x, w_qkv, w_out, b_out):
    out, _ = run(x, w_qkv, w_out, b_out, trace=False)
    return out


# revision 53
# speedup vs baseline: 1.3109x; 1.0400x over previous
"""Distributed multi-head attention kernel for 8 TRN2 NeuronCores.

Problem: x [4, 2048, 1024] -> qkv proj -> 16-head attention (d=64)
         -> out proj + bias -> [4, 2048, 1024].

Sharding (head-split, no collectives): core i handles batch b = i//2 and
head-half hh = i%2 (8 heads, full 2048-token sequence). Each core
computes Q/K/V projections only for its own 8 heads, attention for
those heads, and a partial output projection (+bias on hh=0 cores).
The host sums the two partial outputs per batch.

The kernel is built around the ScalarE exp() bottleneck (33.5M
exp/core; an ACTIVATE costs ~(N+312)/1.2 ns, so exp runs as 256
[128,1024] psum-span instructions at ~1.15us cadence). Everything else
is arranged to never stall ACT:

  - S^T spans are double-buffered (4 psum banks) and emitted two steps
    ahead; after each ACT the S lookahead is emitted BEFORE PV so the
    ACT-critical chain (S -> ACT) stays short.
  - U [65,1024] accumulates PV per (head, q-chunk); row 64 (ones column
    of V) gives softmax denominators free. U is drained to SBUF
    immediately to free its single psum buffer; the normalize
    (K=1 broadcast matmul of D, reciprocal_approx_fast, gpsimd
    multiply into pair-packed UN) runs lazily as filler work.
  - Projections and the output projection run as small filler closures
    (<=4 matmuls) paced by emission deadlines between attention steps.
    Deadlines are also correctness-critical: a tile-writing filler must
    be EMITTED before any reader (the Tile framework only orders reads
    against earlier-emitted writes), so emit_S/PV force-pop exactly the
    closures they depend on via per-(tensor,pair,chunk) counters.
  - Input DMAs are split across the two hardware DGE queues (SP + ACT)
    in need-order; dense warmup matmuls keep the PE HAM clock-gate warm
    while they land (throughput doubles warm: 216 vs 454 ns per 512-col
    stream).
  - Unit order is qc-major within a head-pair so the first token-half's
    output projection + DMA-out streams during the last units.

Known environment hazard: when all 8 cores run simultaneously the
package power/thermal throttle can pin the PE HAM gate at K=4/8
(1.2 GHz) making everything ~25% slower; the schedule stays correct
and near-optimal in either clock regime.
"""

import numpy as np
import ml_dtypes

B = 4
N = 2048
DIM = 1024
HEADS = 16
DH = 64
NCORES = 8
NH = 8       # heads per core
NPAIR = 4    # head pairs per core

_CACHE = {}


def _build_nc():
    from contextlib import ExitStack

    import concourse.bass as bass
    import concourse.mybir as mybir
    import concourse.tile as tile
    from concourse import bacc

    f32 = mybir.dt.float32
    bf16 = mybir.dt.bfloat16
    f16 = mybir.dt.float16
    EXP = mybir.ActivationFunctionType.Exp

    nc = bacc.Bacc("TRN2", target_bir_lowering=False, debug=False,
                   num_devices=NCORES)

    xt_d = nc.dram_tensor("xt", [DIM, N], bf16, kind="ExternalInput")
    wq_d = nc.dram_tensor("wq", [128, 4096], bf16, kind="ExternalInput")
    wk_d = nc.dram_tensor("wk", [128, 4096], bf16, kind="ExternalInput")
    wv_d = nc.dram_tensor("wv", [128, 4096], bf16, kind="ExternalInput")
    wo_d = nc.dram_tensor("wo", [NPAIR, 128, DIM], bf16, kind="ExternalInput")
    bias_d = nc.dram_tensor("bias", [128, DIM], bf16, kind="ExternalInput")
    out_d = nc.dram_tensor("out", [N, DIM], bf16, kind="ExternalOutput")

    with tile.TileContext(nc) as tc, ExitStack() as top:
        const_pool = top.enter_context(tc.tile_pool(name="const", bufs=1))
        s_ps = top.enter_context(tc.tile_pool(name="sps", bufs=2, space="PSUM"))
        u_ps = top.enter_context(tc.tile_pool(name="ups", bufs=1, space="PSUM"))
        mm_ps = top.enter_context(tc.tile_pool(name="mmps", bufs=2, space="PSUM"))
        es_pool = top.enter_context(tc.tile_pool(name="es", bufs=6))
        ur_pool = top.enter_context(tc.tile_pool(name="ur", bufs=4))
        d_pool = top.enter_context(tc.tile_pool(name="dsb", bufs=4))
        r_pool = top.enter_context(tc.tile_pool(name="rsb", bufs=2))
        un_pool = top.enter_context(tc.tile_pool(name="un", bufs=1))

        ones_t = const_pool.tile([1, 128], f16, tag="ones", name="ones")
        nc.gpsimd.memset(ones_t[:], 1.0)
        warm_t = const_pool.tile([128, 512], bf16, tag="warm", name="warm")
        nc.gpsimd.memset(warm_t[:], 0.0)
        bias_t = const_pool.tile([128, DIM], bf16, tag="bias", name="bias")

        # ---- static input tiles -------------------------------------
        # (xt/w innermost: released mid-kernel; pool releases are LIFO)
        qkv_pool = tc.alloc_tile_pool(name="qkv", bufs=1)
        wo_pool = tc.alloc_tile_pool(name="wo", bufs=1)
        xt_pool = tc.alloc_tile_pool(name="xt", bufs=1)
        w_pool = tc.alloc_tile_pool(name="w", bufs=1)
        xt_all = xt_pool.tile([128, 8, N], bf16, tag="xt", name="xt")
        xt = [xt_all[:, i, :] for i in range(8)]
        wq_t = w_pool.tile([128, 4096], bf16, tag="wq", name="wq")
        wk_t = w_pool.tile([128, 4096], bf16, tag="wk", name="wk")
        wv_t = w_pool.tile([128, 4096], bf16, tag="wv", name="wv")
        wq = [wq_t[:, i * 512:(i + 1) * 512] for i in range(8)]
        wk = [wk_t[:, i * 512:(i + 1) * 512] for i in range(8)]
        wv = [wv_t[:, i * 512:(i + 1) * 512] for i in range(8)]
        WO = [wo_pool.tile([128, DIM], bf16, tag=f"wo{p}", name=f"wo{p}")
              for p in range(NPAIR)]

        # DMA split across the two HWDGE queues: SP gets what the first
        # projections need (wk, xt first halves); ACT gets the rest.
        # priority order: the preamble needs wk+xtA (sync) and wv+wq
        # (scalar) only; xtB next; wo/bias are needed ~2/3 in.
        nc.sync.dma_start(wk_t[:], wk_d.ap()[:])
        xt_src = xt_d.ap().rearrange("(i p) n -> p i n", p=128)
        for qtr in range(4):
            nc.sync.dma_start(xt_all[:, :, qtr * 512:(qtr + 1) * 512],
                              xt_src[:, :, qtr * 512:(qtr + 1) * 512])
        for p in range(NPAIR):
            nc.sync.dma_start(WO[p][:], wo_d.ap()[p])
        nc.sync.dma_start(bias_t[:], bias_d.ap()[:])
        nc.scalar.dma_start(wq_t[:], wq_d.ap()[:])
        nc.scalar.dma_start(wv_t[:], wv_d.ap()[:])

        QT = [qkv_pool.tile([128, N], bf16, tag=f"q{p}", name=f"q{p}")
              for p in range(NPAIR)]
        KT = [qkv_pool.tile([128, N], bf16, tag=f"k{p}", name=f"k{p}")
              for p in range(NPAIR)]
        VT = [qkv_pool.tile([128, NH, 65], bf16, tag=f"v{tb}", name=f"v{tb}")
              for tb in range(16)]
        UN = [un_pool.tile([128, N], bf16, tag=f"un{p}", name=f"un{p}")
              for p in range(NPAIR)]

        # ---- PE warmup: keep HAM at K=8/8 while input DMAs land -----
        # dense K=128 matmuls: K=1 streams don't register enough activity
        # to lift the HAM clock gate.
        for i in range(40):
            ps = mm_ps.tile([128, 512], f32, tag="mm", name="wu")
            nc.tensor.matmul(ps[:], warm_t[:, 0:128], warm_t[:],
                             start=True, stop=True)

        # ---- projection unit closures (split into <=4-MM halves) ----
        # pending counters are per (tensor, pair, tchunk) so a force only
        # pulls exactly what an S step needs, not the whole pair.
        pending_kq = {}
        pending_v = {tb: 0 for tb in range(16)}

        def kq_first(box, w, p, t):
            ps = mm_ps.tile([128, 512], f32, tag="mm", name="mm")
            box[0] = ps
            for fc in range(4):
                nc.tensor.matmul(
                    ps[:], w[fc][:, p * 128:(p + 1) * 128],
                    xt[fc][:, t * 512:(t + 1) * 512],
                    start=(fc == 0), stop=False)

        def kq_second(box, dest, w, p, t):
            ps = box[0]
            for fc in range(4, 8):
                nc.tensor.matmul(
                    ps[:], w[fc][:, p * 128:(p + 1) * 128],
                    xt[fc][:, t * 512:(t + 1) * 512],
                    start=False, stop=(fc == 7))
            nc.vector.tensor_copy(dest[p][:, t * 512:(t + 1) * 512], ps[:])

        def kq_unit(dest, w, p, t):
            box = [None]
            kq_first(box, w, p, t)
            kq_second(box, dest, w, p, t)

        def v_first(box, tb):
            ps = mm_ps.tile([128, 512], f32, tag="mm", name="mm")
            box[0] = ps
            for fc in range(4):
                nc.tensor.matmul(
                    ps[:], xt[fc][:, tb * 128:(tb + 1) * 128], wv[fc][:],
                    start=(fc == 0), stop=False)

        def v_second(box, tb):
            ps = box[0]
            for fc in range(4, 8):
                nc.tensor.matmul(
                    ps[:], xt[fc][:, tb * 128:(tb + 1) * 128], wv[fc][:],
                    start=False, stop=(fc == 7))
            nc.vector.tensor_copy(
                VT[tb][:, :, 0:64],
                ps[:].rearrange("p (h d) -> p h d", d=64))
            nc.gpsimd.memset(VT[tb][:, :, 64:65], 1.0)

        def v_unit(tb):
            box = [None]
            v_first(box, tb)
            v_second(box, tb)

        # ---- fillers with emission deadlines ------------------------
        fillers = []
        state = {"emitted": 0, "total": 0}

        def add_filler(latest, fn):
            fillers.append((latest, fn))

        def pop_filler():
            _, fn = fillers.pop(0)
            fn()
            state["emitted"] += 1

        def add_kq_filler(latest, dest, w, p, t):
            box = [None]
            key = (id(dest), p, t)
            pending_kq[key] = pending_kq.get(key, 0) + 2

            def first():
                kq_first(box, w, p, t)
                pending_kq[key] -= 1

            def second():
                kq_second(box, dest, w, p, t)
                pending_kq[key] -= 1

            add_filler(latest, first)
            add_filler(latest, second)

        def add_v_filler(latest, tb):
            box = [None]
            pending_v[tb] += 2

            def first():
                v_first(box, tb)
                pending_v[tb] -= 1

            def second():
                v_second(box, tb)
                pending_v[tb] -= 1

            add_filler(latest, first)
            add_filler(latest, second)

        def maybe_fill(done, steps):
            # deadlines are correctness-critical (a write filler emitted
            # after its reader leaves the reader on stale data): pop
            # through the last due entry; then pace (max 3 per call).
            # deadline-driven only: the spread deadlines already encode a
            # near-uniform ~1 closure / 3 steps feed rate, which keeps
            # the PE ~95% busy (warm HAM clock) without starving ACT.
            due_idx = -1
            for i, (latest, _) in enumerate(fillers):
                if latest is not None and done >= latest:
                    due_idx = i
            while due_idx >= 0:
                pop_filler()
                due_idx -= 1

        def force_keys(keys):
            while any(pending_kq.get(k, 0) > 0 for k in keys):
                pop_filler()

        def force_v(tb):
            while pending_v[tb] > 0:
                pop_filler()

        # preamble: what (pair 0, qc 0) needs before its first steps
        kq_unit(KT, wk, 0, 0)
        kq_unit(QT, wq, 0, 0)
        kq_unit(QT, wq, 0, 1)
        for tb in range(2):
            v_unit(tb)

        # remaining proj as deadline fillers
        add_kq_filler(0, KT, wk, 0, 1)
        for tb in range(2, 16):
            add_v_filler(max(0, tb - 4), tb)
        add_kq_filler(5, KT, wk, 0, 2)
        add_kq_filler(9, KT, wk, 0, 3)
        for t in range(2, 4):
            add_kq_filler(16, QT, wq, 0, t)
        for p in range(1, NPAIR):
            base = 64 * (p - 1) + 16
            for i, (dest, w, t) in enumerate(
                    [(KT, wk, t) for t in range(4)] +
                    [(QT, wq, t) for t in range(4)]):
                add_kq_filler(base + 5 * i, dest, w, p, t)

        # ---- attention, software-pipelined across all 16 units ------
        # qc-major within a pair: both heads' qc=0 first, so the first
        # token-half's output can stream during the last units.
        units = [(p, hh, qc) for p in range(NPAIR) for qc in range(2)
                 for hh in range(2)]
        NU = len(units)
        GTOT = NU * 16

        def emit_S(gidx):
            u, kc = divmod(gidx, 16)
            p, hh, qc = units[u]
            force_keys([(id(KT), p, kc // 4),
                        (id(QT), p, 2 * qc), (id(QT), p, 2 * qc + 1)])
            hb = hh * 64
            st = s_ps.tile([128, 1024], f32, tag="s", name="s")
            for j in range(2):
                nc.tensor.matmul(
                    st[:, j * 512:(j + 1) * 512],
                    KT[p][hb:hb + 64, kc * 128:(kc + 1) * 128],
                    QT[p][hb:hb + 64,
                          qc * 1024 + j * 512:qc * 1024 + j * 512 + 512],
                    start=True, stop=True)
            return st

        def norm_rest(p, hh, qc, ur, dsb):
            """Lazy normalize: broadcast 1/D and multiply into UN."""
            hb = hh * 64
            for j in range(2):
                bc = mm_ps.tile([128, 512], f32, tag="mm", name="bc")
                nc.tensor.matmul(bc[:], ones_t[:],
                                 dsb[:, j * 512:(j + 1) * 512],
                                 start=True, stop=True)
                rs = r_pool.tile([64, 512], f32, tag="rs", name="rs")
                nc.vector.reciprocal_approx_fast(rs[:], bc[0:64, :])
                nc.gpsimd.tensor_mul(
                    UN[p][hb:hb + 64,
                          qc * 1024 + j * 512:qc * 1024 + j * 512 + 512],
                    ur[:, j * 512:(j + 1) * 512], rs[:])

        S_tiles = {0: emit_S(0), 1: emit_S(1)}
        U_box = [None]

        passA_added = [False, False]
        passB_added = [False]
        fin_state = {}

        def setup_fin():
            w_pool.release()
            xt_pool.release()
            fin_state["pool"] = tc.alloc_tile_pool(name="fin", bufs=1)
            fin_state["FIN"] = [
                fin_state["pool"].tile([128, DIM], bf16, tag=f"fin{qf}",
                                       name=f"fin{qf}")
                for qf in range(16)]

        def passA(qf, of):
            FIN = fin_state["FIN"]
            ps = mm_ps.tile([128, 512], f32, tag="mm", name="pa")
            for p in range(3):
                nc.tensor.matmul(
                    ps[:], UN[p][:, qf * 128:(qf + 1) * 128],
                    WO[p][:, of * 512:(of + 1) * 512],
                    start=(p == 0), stop=(p == 2))
            nc.vector.tensor_add(
                FIN[qf][:, of * 512:(of + 1) * 512], ps[:],
                bias_t[:, of * 512:(of + 1) * 512])

        def passB_of(qf, of):
            FIN = fin_state["FIN"]
            ps = mm_ps.tile([128, 512], f32, tag="mm", name="pb")
            nc.tensor.matmul(
                ps[:], UN[3][:, qf * 128:(qf + 1) * 128],
                WO[3][:, of * 512:(of + 1) * 512],
                start=True, stop=True)
            nc.vector.tensor_add(
                FIN[qf][:, of * 512:(of + 1) * 512],
                FIN[qf][:, of * 512:(of + 1) * 512], ps[:])

        def passB_dma(qf):
            nc.sync.dma_start(out_d.ap()[qf * 128:(qf + 1) * 128, :],
                              fin_state["FIN"][qf][:])

        def passB(qf):
            passB_of(qf, 0)
            passB_of(qf, 1)
            passB_dma(qf)

        es_tiles = {}

        def pv_job(gp, gnow):
            """PV for step gp (runs 2 steps late so a late V tile never
            blocks the ACT-critical S chain in the in-order PE queue)."""
            u, kc = divmod(gp, 16)
            p, hh, qc = units[u]
            hloc = 2 * p + hh
            es = es_tiles.pop(gp)
            if kc == 0:
                U_box[0] = u_ps.tile([65, 1024], f32, tag="u", name="u")
            U = U_box[0]
            force_v(kc)
            for j in range(2):
                nc.tensor.matmul(
                    U[:, j * 512:(j + 1) * 512],
                    VT[kc][:, hloc, 0:65],
                    es[:, j * 512:(j + 1) * 512],
                    start=(kc == 0), stop=(kc == 15))
            if kc == 15:
                # fast U drain: D row first (it gates the bc->recip->mul
                # chain), then the U rows; frees the U psum buffer ASAP.
                dsb = d_pool.tile([1, 1024], f16, tag="d", name="d")
                with nc.allow_low_precision(reason="softmax denom f16"):
                    nc.vector.tensor_copy(dsb[:], U[64:65, :])
                ur = ur_pool.tile([64, 1024], bf16, tag="ur", name="ur")
                nc.vector.tensor_copy(ur[:], U[0:64, :])
                add_filler(min(gnow + 10, 250),
                           lambda p=p, hh=hh, qc=qc, ur=ur, dsb=dsb:
                           norm_rest(p, hh, qc, ur, dsb))
                if u == 9 and not passA_added[0]:
                    # pairs 0-2 qc0 done: out-proj for tokens 0-1023
                    passA_added[0] = True
                    setup_fin()
                    for i, (qf, of) in enumerate(
                            (qf, of) for qf in range(8) for of in range(2)):
                        add_filler(gnow + 12 + i * 3,
                                   lambda qf=qf, of=of: passA(qf, of))
                if u == 11 and not passA_added[1]:
                    passA_added[1] = True
                    for i, (qf, of) in enumerate(
                            (qf, of) for qf in range(8, 16) for of in range(2)):
                        add_filler(gnow + 12 + i * 2,
                                   lambda qf=qf, of=of: passA(qf, of))
                if u == 13 and not passB_added[0]:
                    passB_added[0] = True
                    i = 0
                    for qf in range(8):
                        for of in range(2):
                            add_filler(gnow + 12 + i,
                                       lambda qf=qf, of=of: passB_of(qf, of))
                            i += 1
                        add_filler(gnow + 12 + i,
                                   lambda qf=qf: passB_dma(qf))

        for gidx in range(GTOT):
            st = S_tiles.pop(gidx)
            es = es_pool.tile([128, 1024], bf16, tag="es", name="es")
            nc.scalar.activation(es[:], st[:], EXP, scale=0.125)
            es_tiles[gidx] = es
            # S lookahead FIRST: it feeds ACT(g+2), the critical chain.
            if gidx + 2 < GTOT:
                S_tiles[gidx + 2] = emit_S(gidx + 2)
            if gidx - 2 >= 0:
                pv_job(gidx - 2, gidx)
            maybe_fill(gidx, GTOT)

        pv_job(GTOT - 2, GTOT - 1)
        pv_job(GTOT - 1, GTOT - 1)

        # flush remaining fillers (incl. last norms and any passA/B)
        while fillers:
            pop_filler()

        # tail: second token-half out-proj + DMA
        for qf in range(8, 16):
            passB(qf)

        fin_state["pool"].release()
        wo_pool.release()
        qkv_pool.release()

    nc.compile()
    return nc


def _get_nc():
    if "nc" not in _CACHE:
        _CACHE["nc"] = _build_nc()
    return _CACHE["nc"]


def _make_in_maps(x, w_qkv, w_out, b_out):
    bf = ml_dtypes.bfloat16

    def wslice(w, hh):
        # [1024, 512] -> [128, 8, 512] (partition p holds w[fc*128+p, :]
        # at slot fc) -> [128, 4096]
        s = np.asarray(w[:, hh * 512:(hh + 1) * 512], np.float32)
        return np.ascontiguousarray(
            s.reshape(8, 128, 512).transpose(1, 0, 2).reshape(128, 4096)
        ).astype(bf)

    xts = [np.ascontiguousarray(np.asarray(x[b], np.float32).T).astype(bf)
           for b in range(B)]
    wq_f = w_qkv[:, 0:1024]
    wk_f = w_qkv[:, 1024:2048]
    wv_f = w_qkv[:, 2048:3072]
    wo_f = np.asarray(w_out, np.float32)  # [1024 inner, 1024 out]
    bias_rep = np.broadcast_to(
        np.asarray(b_out, np.float32).reshape(1, DIM), (128, DIM))
    zeros = np.zeros((128, DIM), np.float32)
    in_maps = []
    for i in range(NCORES):
        b, hh = i // 2, i % 2
        wo_core = np.ascontiguousarray(
            wo_f[hh * 512:(hh + 1) * 512, :]).reshape(NPAIR, 128, DIM)
        in_maps.append({
            "xt": xts[b],
            "wq": wslice(wq_f, hh),
            "wk": wslice(wk_f, hh),
            "wv": wslice(wv_f, hh),
            "wo": wo_core.astype(bf),
            "bias": np.ascontiguousarray(
                (bias_rep if hh == 0 else zeros)).astype(bf),
        })
    return in_maps


def _assemble(results):
    out = np.empty((B, N, DIM), np.float32)
    for b in range(B):
        out[b] = (results[2 * b]["out"].astype(np.float32) +
                  results[2 * b + 1]["out"].astype(np.float32))
    return out


def run(x, w_qkv, w_out, b_out, trace=False):
    """Run the kernel; returns (output, BassKernelResults)."""
    from concourse.bass_utils import run_bass_kernel_spmd
    nc = _get_nc()
    in_maps = _make_in_maps(x, w_qkv, w_out, b_out)
    res = run_bass_kernel_spmd(nc, in_maps, core_ids=list(range(NCORES)),
                               trace=trace)
    return _assemble(res.results), res


def kernel(x, w_qkv, w_out, b_out):
    out, _ = run(x, w_qkv, w_out, b_out, trace=False)
    return out
